# revision 14
# baseline (speedup 1.0000x reference)
"""Bayesian MLP forward on 8 Trainium2 NeuronCores.

Strategy: 8-way model parallelism over the hidden dimension (512 output
features per core), feature-major activations [feat, batch] so no
transposes are needed on device. bf16 matmuls with f32 PSUM
accumulation.

Cross-core exchange is ONE fused AllGather: each core contributes its
unnormalized y1 shard (bf16) plus its w1/w2 sum-of-squares partials
packed into an extra row; every core then reduces the partials locally,
forms the vMF weight norms, and applies scale+bias+relu to the full
gathered h1. Layer 3 is sharded over the contraction dim and finished
with a 20KB AllReduce, then log-softmax over the 5 classes is computed
batch-major ([128 x 8 x 5]) so nothing runs partition-starved. TRN2's
ACT engine has no Ln table, so ln() is evaluated on the vector engine
from exponent/mantissa bits plus an atanh-series polynomial.
"""

import math

import numpy as np

N_CORES = 8
B, IN, H, OUT = 1024, 4096, 4096, 5
SH = H // N_CORES  # 512 features per core
KT = IN // 128  # 32 contraction tiles (same for layer 2)
OJ = SH // 128  # 4 output-feature tiles per core
NBB = B // 512  # 2 batch halves of 512
NG = B // 128  # 8 batch groups for the softmax tail
AGR = SH + 1  # rows per rank in the fused AllGather (512 data + 1 meta)

NHLOG2PI = -0.5 * math.log(2.0 * math.pi)
LN2 = math.log(2.0)

_CACHE = {}


def _emit_ln(nc, pool, mybir, out_ap, in_ap, shape, pref):
    """out = ln(in) elementwise for positive f32 input, on the DVE.

    x = 2^e * m, m in [1,2): ln x = e*ln2 + 2*atanh((m-1)/(m+1)) with the
    atanh series in z = s^2 truncated at z^5 (|s| <= 1/3 so the
    truncation error is ~2.5e-8).
    """
    f32 = mybir.dt.float32
    u32 = mybir.dt.uint32
    op = mybir.AluOpType
    xi = in_ap.bitcast(u32)

    eu = pool.tile(shape, u32, tag=f"{pref}_eu", name=f"{pref}_eu")
    nc.vector.tensor_scalar(eu[:], xi, 23, None, op0=op.logical_shift_right)
    ef = pool.tile(shape, f32, tag=f"{pref}_ef", name=f"{pref}_ef")
    nc.vector.tensor_copy(ef[:], eu[:])
    et = pool.tile(shape, f32, tag=f"{pref}_et", name=f"{pref}_et")
    nc.vector.tensor_scalar(
        et[:], ef[:], LN2, 127.0 * LN2, op0=op.mult, op1=op.subtract
    )

    mu = pool.tile(shape, u32, tag=f"{pref}_mu", name=f"{pref}_mu")
    nc.vector.tensor_scalar(
        mu[:], xi, 0x007FFFFF, 0x3F800000, op0=op.bitwise_and, op1=op.bitwise_or
    )
    m = mu[:].bitcast(f32)
    num = pool.tile(shape, f32, tag=f"{pref}_num", name=f"{pref}_num")
    nc.vector.tensor_scalar(num[:], m, 1.0, None, op0=op.subtract)
    den = pool.tile(shape, f32, tag=f"{pref}_den", name=f"{pref}_den")
    nc.vector.tensor_scalar(den[:], m, 1.0, None, op0=op.add)
    nc.vector.reciprocal(den[:], den[:])
    s = pool.tile(shape, f32, tag=f"{pref}_s", name=f"{pref}_s")
    nc.vector.tensor_mul(s[:], num[:], den[:])
    z = pool.tile(shape, f32, tag=f"{pref}_z", name=f"{pref}_z")
    nc.vector.tensor_mul(z[:], s[:], s[:])
    p = pool.tile(shape, f32, tag=f"{pref}_p", name=f"{pref}_p")
    nc.vector.tensor_scalar(
        p[:], z[:], 1.0 / 11.0, 1.0 / 9.0, op0=op.mult, op1=op.add
    )
    for c in (1.0 / 7.0, 1.0 / 5.0, 1.0 / 3.0):
        nc.vector.tensor_mul(p[:], p[:], z[:])
        nc.vector.tensor_scalar(p[:], p[:], c, None, op0=op.add)
    nc.vector.tensor_mul(p[:], p[:], z[:])  # Q = z*poly
    nc.vector.tensor_scalar(p[:], p[:], 1.0, None, op0=op.add)  # 1+Q
    nc.vector.tensor_mul(p[:], p[:], s[:])  # s*(1+Q)
    nc.vector.tensor_scalar(p[:], p[:], 2.0, None, op0=op.mult)  # ln(m)
    nc.vector.tensor_add(out_ap, p[:], et[:])


def _build_nc():
    import concourse.bass as bass  # noqa: F401
    import concourse.bass_isa as bass_isa
    import concourse.mybir as mybir
    import concourse.tile as tile
    from concourse import bacc

    f32 = mybir.dt.float32
    bf16 = mybir.dt.bfloat16
    AF = mybir.ActivationFunctionType
    op = mybir.AluOpType
    RED = bass_isa.ReduceOp
    AX = mybir.AxisListType
    RG = [list(range(N_CORES))]

    nc = bacc.Bacc(
        "TRN2", target_bir_lowering=False, debug=False, num_devices=N_CORES
    )

    # ---- kernel I/O ----
    xT = nc.dram_tensor("xT", [IN, B], bf16, kind="ExternalInput")
    w1s = nc.dram_tensor("w1s", [IN, SH], bf16, kind="ExternalInput")
    w2s = nc.dram_tensor("w2s", [H, SH], bf16, kind="ExternalInput")
    b1c = nc.dram_tensor("b1c", [128, KT], f32, kind="ExternalInput")
    b2f = nc.dram_tensor("b2f", [128, H // 128], f32, kind="ExternalInput")
    b2s = nc.dram_tensor("b2s", [SH], f32, kind="ExternalInput")
    w3s = nc.dram_tensor("w3s", [SH, OUT], f32, kind="ExternalInput")
    w3f = nc.dram_tensor("w3f", [128, (OUT * H) // 128], f32, kind="ExternalInput")
    rw3 = nc.dram_tensor("rw3", [128, (OUT * H) // 128], f32, kind="ExternalInput")
    b3r = nc.dram_tensor("b3r", [1, OUT], f32, kind="ExternalInput")
    rb3 = nc.dram_tensor("rb3", [1, OUT], f32, kind="ExternalInput")
    outB = nc.dram_tensor("outB", [B, OUT], f32, kind="ExternalOutput")
    scal = nc.dram_tensor("scal", [1, 8], f32, kind="ExternalOutput")

    with tile.TileContext(nc) as tc:
        with (
            tc.tile_pool(name="const", bufs=1) as const,
            tc.tile_pool(name="lnp", bufs=1) as lnp,
            tc.tile_pool(name="hst", bufs=3) as hstp,
            tc.tile_pool(name="acts", bufs=1) as acts,
            tc.tile_pool(name="psum", bufs=8, space="PSUM") as psum,
            tc.tile_pool(name="dram", bufs=1, space="DRAM") as dram,
        ):
            FR = (OUT * H) // 128  # 160

            # ---------- stream w1 + x (both bf16), then w2 ----------
            w1b = const.tile([128, KT * SH], bf16, name="w1b")
            xb = acts.tile([128, KT * B], bf16, tag="acts", name="xb")
            ss1c = const.tile([128, KT], f32, name="ss1c")
            sq_scr = const.tile([128, 512], f32, name="sq_scr")
            for t in range(KT):
                nc.sync.dma_start(
                    w1b[:, t * SH : (t + 1) * SH],
                    w1s.ap()[t * 128 : (t + 1) * 128, :],
                )
                nc.scalar.activation(
                    sq_scr[:],
                    w1b[:, t * SH : (t + 1) * SH],
                    AF.Square,
                    accum_out=ss1c[:, t : t + 1],
                )
                nc.sync.dma_start(
                    xb[:, t * B : (t + 1) * B],
                    xT.ap()[t * 128 : (t + 1) * 128, :],
                )

            w2b = const.tile([128, KT * SH], bf16, name="w2b")
            ss2c = const.tile([128, KT], f32, name="ss2c")
            for t in range(KT):
                nc.sync.dma_start(
                    w2b[:, t * SH : (t + 1) * SH],
                    w2s.ap()[t * 128 : (t + 1) * 128, :],
                )
                nc.scalar.activation(
                    sq_scr[:],
                    w2b[:, t * SH : (t + 1) * SH],
                    AF.Square,
                    accum_out=ss2c[:, t : t + 1],
                )

            # ---------- GEMM1 (t-outer: overlaps with the input DMA stream) ----------
            ps1 = [
                psum.tile([128, 512], f32, tag="ps", name=f"ps1_{g}")
                for g in range(OJ * NBB)
            ]
            for t in range(KT):
                for j in range(OJ):
                    for bb in range(NBB):
                        nc.tensor.matmul(
                            ps1[j * NBB + bb][:],
                            w1b[:, t * SH + j * 128 : t * SH + (j + 1) * 128],
                            xb[:, t * B + bb * 512 : t * B + (bb + 1) * 512],
                            start=(t == 0),
                            stop=(t == KT - 1),
                        )
            # evacuate unnormalized y1 as bf16 (normalize after the AllGather)
            y1sl = const.tile([128, OJ * B], bf16, name="y1sl")
            for j in range(OJ):
                for bb in range(NBB):
                    nc.vector.tensor_copy(
                        y1sl[:, j * B + bb * 512 : j * B + (bb + 1) * 512],
                        ps1[j * NBB + bb][:],
                    )

            # ---------- w-norm partials: two tiny early AllReduces ----------
            ss1p = const.tile([128, 1], f32, name="ss1p")
            nc.vector.reduce_sum(ss1p[:], ss1c[:], axis=AX.X)
            ss1a = const.tile([128, 1], f32, name="ss1a")
            nc.gpsimd.partition_all_reduce(
                ss1a[:], ss1p[:], channels=128, reduce_op=RED.add
            )
            ar1_in = dram.tile([1, 1], f32, name="ar1_in")
            ar1_out = dram.tile([1, 1], f32, name="ar1_out")
            nc.sync.dma_start(ar1_in[:], ss1a[:1, :])
            nc.gpsimd.collective_compute(
                "AllReduce",
                op.add,
                replica_groups=RG,
                ins=[ar1_in.opt()],
                outs=[ar1_out.opt()],
            )
            ss2p = const.tile([128, 1], f32, name="ss2p")
            nc.vector.reduce_sum(ss2p[:], ss2c[:], axis=AX.X)
            ss2a = const.tile([128, 1], f32, name="ss2a")
            nc.gpsimd.partition_all_reduce(
                ss2a[:], ss2p[:], channels=128, reduce_op=RED.add
            )
            ar2_in = dram.tile([1, 1], f32, name="ar2_in")
            ar2_out = dram.tile([1, 1], f32, name="ar2_out")
            nc.sync.dma_start(ar2_in[:], ss2a[:1, :])
            nc.gpsimd.collective_compute(
                "AllReduce",
                op.add,
                replica_groups=RG,
                ins=[ar2_in.opt()],
                outs=[ar2_out.opt()],
            )

            # ---------- AllGather of the unnormalized y1 shard (bf16) ----------
            ag1_in = dram.tile([SH, B], bf16, name="ag1_in")
            ag1_out = dram.tile(
                [N_CORES * SH, B], bf16, name="ag1_out", addr_space="Shared"
            )
            for j in range(OJ):
                nc.sync.dma_start(
                    ag1_in[j * 128 : (j + 1) * 128, :],
                    y1sl[:, j * B : (j + 1) * B],
                )
            nc.gpsimd.collective_compute(
                "AllGather",
                op.bypass,
                replica_groups=RG,
                ins=[ag1_in.opt()],
                outs=[ag1_out.opt()],
            )

            # ---------- small constants / scalar reductions ----------
            # (emitted after the heavy streams so they don't hog engines early;
            # all Exp ops precede every Sqrt so the ACT table switches just twice)
            rw3t = const.tile([128, FR], f32, name="rw3t")
            nc.sync.dma_start(rw3t[:], rw3.ap())
            nc.scalar.activation(rw3t[:], rw3t[:], AF.Exp)
            nc.vector.tensor_scalar(rw3t[:], rw3t[:], 1.0, None, op0=op.add)
            sw3 = const.tile([128, FR], f32, name="sw3")
            _emit_ln(nc, lnp, mybir, sw3[:], rw3t[:], [128, FR], "lnA")
            _emit_ln(nc, lnp, mybir, sw3[:], sw3[:], [128, FR], "lnB")
            slwp = const.tile([128, 1], f32, name="slwp")
            nc.vector.reduce_sum(slwp[:], sw3[:], axis=AX.X)
            slwa = const.tile([128, 1], f32, name="slwa")
            nc.gpsimd.partition_all_reduce(
                slwa[:], slwp[:], channels=128, reduce_op=RED.add
            )

            rb3t = const.tile([1, OUT], f32, name="rb3t")
            nc.sync.dma_start(rb3t[:], rb3.ap())
            nc.scalar.activation(rb3t[:], rb3t[:], AF.Exp)
            nc.vector.tensor_scalar(rb3t[:], rb3t[:], 1.0, None, op0=op.add)
            sb3 = const.tile([1, OUT], f32, name="sb3")
            _emit_ln(nc, lnp, mybir, sb3[:], rb3t[:], [1, OUT], "lnC")
            _emit_ln(nc, lnp, mybir, sb3[:], sb3[:], [1, OUT], "lnD")
            slb3 = const.tile([1, 1], f32, name="slb3")
            nc.vector.reduce_sum(slb3[:], sb3[:], axis=AX.X)

            # b1 / b2 norms (full vectors, locally on every core)
            b1t = const.tile([128, KT], f32, name="b1t")
            nc.sync.dma_start(b1t[:], b1c.ap())
            ssb1p = const.tile([128, 1], f32, name="ssb1p")
            nc.scalar.activation(
                sq_scr[:, :KT], b1t[:], AF.Square, accum_out=ssb1p[:]
            )
            ssb1a = const.tile([128, 1], f32, name="ssb1a")
            nc.gpsimd.partition_all_reduce(
                ssb1a[:], ssb1p[:], channels=128, reduce_op=RED.add
            )
            invb1 = const.tile([128, 1], f32, name="invb1")
            nc.vector.reciprocal(invb1[:], ssb1a[:])
            nc.scalar.sqrt(invb1[:], invb1[:])  # 1/||b1||
            mub1_all = const.tile([128, KT], f32, name="mub1_all")
            nc.vector.tensor_scalar(
                mub1_all[:], b1t[:], invb1[:, 0:1], None, op0=op.mult
            )

            b2t = const.tile([128, H // 128], f32, name="b2t")
            nc.sync.dma_start(b2t[:], b2f.ap())
            ssb2p = const.tile([128, 1], f32, name="ssb2p")
            nc.scalar.activation(
                sq_scr[:, : H // 128], b2t[:], AF.Square, accum_out=ssb2p[:]
            )
            ssb2a = const.tile([128, 1], f32, name="ssb2a")
            nc.gpsimd.partition_all_reduce(
                ssb2a[:], ssb2p[:], channels=128, reduce_op=RED.add
            )
            invb2 = const.tile([128, 1], f32, name="invb2")
            nc.vector.reciprocal(invb2[:], ssb2a[:])
            nc.scalar.sqrt(invb2[:], invb2[:])
            mub2 = []
            for j in range(OJ):
                t2 = const.tile([128, 1], f32, name=f"b2sl{j}")
                nc.sync.dma_start(t2[:], b2s.ap()[j * 128 : (j + 1) * 128][:, None])
                m2 = const.tile([128, 1], f32, name=f"mub2_{j}")
                nc.vector.tensor_mul(m2[:], t2[:], invb2[:])
                mub2.append(m2)

            # layer-3 weights (feature-sliced rows of w3^T) -> bf16
            w3b = []
            for j in range(OJ):
                t3 = const.tile([128, OUT], f32, name=f"w3st{j}")
                nc.sync.dma_start(t3[:], w3s.ap()[j * 128 : (j + 1) * 128, :])
                wb = const.tile([128, OUT], bf16, name=f"w3b{j}")
                nc.vector.tensor_copy(wb[:], t3[:])
                w3b.append(wb)

            # scalar reductions of layer-3 means (for the log-prior)
            w3t = const.tile([128, FR], f32, name="w3t")
            nc.sync.dma_start(w3t[:], w3f.ap())
            ssw3p = const.tile([128, 1], f32, name="ssw3p")
            nc.scalar.activation(
                sq_scr[:, :FR], w3t[:], AF.Square, accum_out=ssw3p[:]
            )
            ssw3a = const.tile([128, 1], f32, name="ssw3a")
            nc.gpsimd.partition_all_reduce(
                ssw3a[:], ssw3p[:], channels=128, reduce_op=RED.add
            )
            b3rt = const.tile([1, OUT], f32, name="b3rt")
            nc.sync.dma_start(b3rt[:], b3r.ap())
            ssb3 = const.tile([1, 1], f32, name="ssb3")
            nc.scalar.activation(
                sq_scr[:1, :OUT], b3rt[:], AF.Square, accum_out=ssb3[:]
            )

            # softmax bias pattern [1, NG*OUT] -> broadcast to 128 partitions
            bias40p = const.tile([1, NG * OUT], f32, name="bias40p")
            for g in range(NG):
                nc.vector.tensor_copy(bias40p[:, g * OUT : (g + 1) * OUT], b3rt[:])
            bias40 = const.tile([128, NG * OUT], f32, name="bias40")
            nc.gpsimd.partition_broadcast(bias40[:], bias40p[:], channels=128)

            # ---------- global norms from the AllReduce results ----------
            gss = const.tile([1, 2], f32, name="gss")
            nc.sync.dma_start(gss[:, 0:1], ar1_out[:])
            nc.sync.dma_start(gss[:, 1:2], ar2_out[:])
            invw = const.tile([1, 2], f32, name="invw")
            nc.vector.reciprocal(invw[:], gss[:])
            nc.scalar.sqrt(invw[:], invw[:])  # [1/||w1||, 1/||w2||]
            inv1b = const.tile([128, 1], f32, name="inv1b")
            nc.gpsimd.partition_broadcast(inv1b[:], invw[:, 0:1], channels=128)
            inv2b = const.tile([128, 1], f32, name="inv2b")
            nc.gpsimd.partition_broadcast(inv2b[:], invw[:, 1:2], channels=128)

            # h1 = relu(y1 * inv1 + mu_b1), full 4096 features on every core
            h1b = acts.tile([128, KT * B], bf16, tag="acts", name="h1b")
            for t in range(KT):
                hst = hstp.tile([128, B], bf16, tag="hst", name=f"hst{t}")
                nc.sync.dma_start(
                    hst[:], ag1_out[t * 128 : (t + 1) * 128, :]
                )
                nc.scalar.activation(
                    h1b[:, t * B : (t + 1) * B],
                    hst[:],
                    AF.Relu,
                    bias=mub1_all[:, t : t + 1],
                    scale=inv1b[:],
                )

            # ---------- GEMM2 ----------
            ps2 = [
                psum.tile([128, 512], f32, tag="ps", name=f"ps2_{g}")
                for g in range(OJ * NBB)
            ]
            for t in range(KT):
                for j in range(OJ):
                    for bb in range(NBB):
                        nc.tensor.matmul(
                            ps2[j * NBB + bb][:],
                            w2b[:, t * SH + j * 128 : t * SH + (j + 1) * 128],
                            h1b[:, t * B + bb * 512 : t * B + (bb + 1) * 512],
                            start=(t == 0),
                            stop=(t == KT - 1),
                        )
            h2sl = const.tile([128, OJ * B], bf16, name="h2sl")
            for j in range(OJ):
                for bb in range(NBB):
                    nc.scalar.activation(
                        h2sl[:, j * B + bb * 512 : j * B + (bb + 1) * 512],
                        ps2[j * NBB + bb][:],
                        AF.Relu,
                        bias=mub2[j][:],
                        scale=inv2b[:],
                    )

            # ---------- layer 3 (contraction-sharded) + AllReduce ----------
            ps3 = [
                psum.tile([OUT, 512], f32, tag="ps", name=f"ps3_{bb}")
                for bb in range(NBB)
            ]
            for j in range(OJ):
                for bb in range(NBB):
                    nc.tensor.matmul(
                        ps3[bb][:],
                        w3b[j][:],
                        h2sl[:, j * B + bb * 512 : j * B + (bb + 1) * 512],
                        start=(j == 0),
                        stop=(j == OJ - 1),
                    )
            ar3_in = dram.tile([OUT, B], f32, name="ar3_in")
            ar3_out = dram.tile([OUT, B], f32, name="ar3_out", addr_space="Shared")
            y3p = const.tile([OUT, B], f32, name="y3p")
            for bb in range(NBB):
                nc.vector.tensor_copy(
                    y3p[:, bb * 512 : (bb + 1) * 512], ps3[bb][:]
                )
            nc.sync.dma_start(ar3_in[:], y3p[:])
            nc.gpsimd.collective_compute(
                "AllReduce",
                op.add,
                replica_groups=RG,
                ins=[ar3_in.opt()],
                outs=[ar3_out.opt()],
            )

            # ---------- log-softmax, batch-major [128, NG, OUT] ----------
            y3r = const.tile([128, NG * OUT], f32, name="y3r")
            for g in range(NG):
                nc.sync.dma_start(
                    y3r[:, g * OUT : (g + 1) * OUT],
                    ar3_out[:, g * 128 : (g + 1) * 128].rearrange("c p -> p c"),
                )
            nc.vector.tensor_add(y3r[:], y3r[:], bias40[:])
            # |y3| is O(1) here, so exp() is safe without max-subtraction
            ex = const.tile([128, NG * OUT], f32, name="ex")
            nc.scalar.activation(ex[:], y3r[:], AF.Exp)
            sm = const.tile([128, NG], f32, name="sm")
            nc.vector.reduce_sum(
                sm[:], ex[:].rearrange("p (g c) -> p g c", g=NG), axis=AX.X
            )
            ls = const.tile([128, NG], f32, name="ls")
            _emit_ln(nc, lnp, mybir, ls[:], sm[:], [128, NG], "lnS")
            for g in range(NG):
                nc.vector.tensor_scalar(
                    y3r[:, g * OUT : (g + 1) * OUT],
                    y3r[:, g * OUT : (g + 1) * OUT],
                    ls[:, g : g + 1],
                    None,
                    op0=op.subtract,
                )
            nc.sync.dma_start(
                outB.ap().rearrange("(g p) c -> p g c", p=128),
                y3r[:].rearrange("p (g c) -> p g c", g=NG),
            )

            # ---------- scalar outputs ----------
            scal_sb = const.tile([1, 8], f32, name="scal_sb")
            nc.vector.tensor_copy(scal_sb[:, 0:1], gss[:, 0:1])
            nc.vector.tensor_copy(scal_sb[:, 1:2], gss[:, 1:2])
            nc.vector.tensor_copy(scal_sb[:, 2:3], ssb1a[:1, :])
            nc.vector.tensor_copy(scal_sb[:, 3:4], ssb2a[:1, :])
            nc.vector.tensor_copy(scal_sb[:, 4:5], slwa[:1, :])
            nc.vector.tensor_copy(scal_sb[:, 5:6], slb3[:])
            nc.vector.tensor_copy(scal_sb[:, 6:7], ssw3a[:1, :])
            nc.vector.tensor_copy(scal_sb[:, 7:8], ssb3[:])
            nc.sync.dma_start(scal.ap(), scal_sb[:])

    nc.compile()
    return nc


def _log_surface_area(d):
    h = (d + 1.0) / 2.0
    return math.log(2.0) + h * math.log(math.pi) - math.lgamma(h)


def _log_besseli(s, kappa):
    x = kappa / s
    sq = math.sqrt(1.0 + x * x)
    eta = sq + math.log(x) - math.log1p(sq)
    return s * eta - 0.5 * math.log(2.0 * math.pi * s) - 0.5 * math.log(sq)


def _log_C_vmf(d, kappa):
    s = 0.5 * d - 1.0
    return d * NHLOG2PI + s * math.log(kappa) - _log_besseli(s, kappa)


LAST_RESULTS = None


def kernel(
    x,
    w1_mu,
    w1_logkappa,
    b1_mu,
    b1_logkappa,
    w2_mu,
    w2_logkappa,
    b2_mu,
    b2_logkappa,
    w3_mu,
    w3_rho,
    b3_mu,
    b3_rho,
):
    global LAST_RESULTS
    import ml_dtypes

    from concourse import bass_utils

    if "nc" not in _CACHE:
        _CACHE["nc"] = _build_nc()
    nc = _CACHE["nc"]

    f = np.float32
    x = np.asarray(x, f)
    W1 = np.asarray(w1_mu, f).reshape(IN, H).astype(ml_dtypes.bfloat16)
    W2 = np.asarray(w2_mu, f).reshape(H, H).astype(ml_dtypes.bfloat16)
    b1 = np.ascontiguousarray(np.asarray(b1_mu, f))
    b2 = np.ascontiguousarray(np.asarray(b2_mu, f))
    w3 = np.asarray(w3_mu, f)
    w3T = np.ascontiguousarray(w3.T)
    b3 = np.ascontiguousarray(np.asarray(b3_mu, f))
    xTc = np.ascontiguousarray(x.T).astype(ml_dtypes.bfloat16)
    b1cols = np.ascontiguousarray(b1.reshape(KT, 128).T)
    w3flat = np.ascontiguousarray(w3.reshape(128, (OUT * H) // 128))
    rw3m = np.ascontiguousarray(np.asarray(w3_rho, f).reshape(128, (OUT * H) // 128))
    b3row = b3.reshape(1, OUT)
    rb3m = np.ascontiguousarray(np.asarray(b3_rho, f).reshape(1, OUT))

    in_maps = []
    for c in range(N_CORES):
        sl = slice(c * SH, (c + 1) * SH)
        in_maps.append(
            {
                "xT": xTc,
                "w1s": np.ascontiguousarray(W1[:, sl]),
                "w2s": np.ascontiguousarray(W2[:, sl]),
                "b1c": b1cols,
                "b2f": b2.reshape(128, H // 128),
                "b2s": np.ascontiguousarray(b2[sl]),
                "w3s": np.ascontiguousarray(w3T[sl, :]),
                "w3f": w3flat,
                "rw3": rw3m,
                "b3r": b3row,
                "rb3": rb3m,
            }
        )

    res = bass_utils.run_bass_kernel_spmd(nc, in_maps, core_ids=list(range(N_CORES)))
    LAST_RESULTS = res
    r0 = res.results[0]
    out = np.ascontiguousarray(r0["outB"].astype(np.float32))
    s = r0["scal"][0].astype(np.float64)

    kw1 = math.exp(float(np.float32(w1_logkappa))) + 1e-6
    kb1 = math.exp(float(np.float32(b1_logkappa))) + 1e-6
    kw2 = math.exp(float(np.float32(w2_logkappa))) + 1e-6
    kb2 = math.exp(float(np.float32(b2_logkappa))) + 1e-6
    d_w = float(H * IN)
    d_b = float(H)
    n3 = OUT * H + OUT  # 20485 gaussian params

    lvp = (
        kw1
        + _log_C_vmf(d_w, kw1)
        + kb1
        + _log_C_vmf(d_b, kb1)
        + kw2
        + _log_C_vmf(d_w, kw2)
        + kb2
        + _log_C_vmf(d_b, kb2)
        + n3 * NHLOG2PI
        - (s[4] + s[5])
    )
    lp = -4.0 * _log_surface_area(d_w) + n3 * NHLOG2PI - 0.5 * (s[6] + s[7])

    return out, np.asarray(lvp, np.float32), np.asarray(lp, np.float32)


# revision 17
# speedup vs baseline: 1.0223x; 1.0223x over previous
"""Bayesian MLP forward on 8 Trainium2 NeuronCores.

Strategy: 8-way model parallelism over the hidden dimension (512 output
features per core), feature-major activations [feat, batch] so no
transposes are needed on device. bf16 matmuls with f32 PSUM
accumulation.

Cross-core exchange is ONE fused AllGather: each core contributes its
unnormalized y1 shard (bf16) plus its w1/w2 sum-of-squares partials
packed into an extra row; every core then reduces the partials locally,
forms the vMF weight norms, and applies scale+bias+relu to the full
gathered h1. Layer 3 is sharded over the contraction dim and finished
with a 20KB AllReduce, then log-softmax over the 5 classes is computed
batch-major ([128 x 8 x 5]) so nothing runs partition-starved. TRN2's
ACT engine has no Ln table, so ln() is evaluated on the vector engine
from exponent/mantissa bits plus an atanh-series polynomial.
"""

import math

import numpy as np

N_CORES = 8
B, IN, H, OUT = 1024, 4096, 4096, 5
SH = H // N_CORES  # 512 features per core
KT = IN // 128  # 32 contraction tiles (same for layer 2)
OJ = SH // 128  # 4 output-feature tiles per core
NBB = B // 512  # 2 batch halves of 512
NG = B // 128  # 8 batch groups for the softmax tail
AGR = SH + 1  # rows per rank in the fused AllGather (512 data + 1 meta)

NHLOG2PI = -0.5 * math.log(2.0 * math.pi)
LN2 = math.log(2.0)

_CACHE = {}


def _emit_ln(nc, pool, mybir, out_ap, in_ap, shape, pref):
    """out = ln(in) elementwise for positive f32 input, on the DVE.

    x = 2^e * m, m in [1,2): ln x = e*ln2 + 2*atanh((m-1)/(m+1)) with the
    atanh series in z = s^2 truncated at z^5 (|s| <= 1/3 so the
    truncation error is ~2.5e-8).
    """
    f32 = mybir.dt.float32
    u32 = mybir.dt.uint32
    op = mybir.AluOpType
    xi = in_ap.bitcast(u32)

    eu = pool.tile(shape, u32, tag=f"{pref}_eu", name=f"{pref}_eu")
    nc.vector.tensor_scalar(eu[:], xi, 23, None, op0=op.logical_shift_right)
    ef = pool.tile(shape, f32, tag=f"{pref}_ef", name=f"{pref}_ef")
    nc.vector.tensor_copy(ef[:], eu[:])
    et = pool.tile(shape, f32, tag=f"{pref}_et", name=f"{pref}_et")
    nc.vector.tensor_scalar(
        et[:], ef[:], LN2, 127.0 * LN2, op0=op.mult, op1=op.subtract
    )

    mu = pool.tile(shape, u32, tag=f"{pref}_mu", name=f"{pref}_mu")
    nc.vector.tensor_scalar(
        mu[:], xi, 0x007FFFFF, 0x3F800000, op0=op.bitwise_and, op1=op.bitwise_or
    )
    m = mu[:].bitcast(f32)
    num = pool.tile(shape, f32, tag=f"{pref}_num", name=f"{pref}_num")
    nc.vector.tensor_scalar(num[:], m, 1.0, None, op0=op.subtract)
    den = pool.tile(shape, f32, tag=f"{pref}_den", name=f"{pref}_den")
    nc.vector.tensor_scalar(den[:], m, 1.0, None, op0=op.add)
    nc.vector.reciprocal(den[:], den[:])
    s = pool.tile(shape, f32, tag=f"{pref}_s", name=f"{pref}_s")
    nc.vector.tensor_mul(s[:], num[:], den[:])
    z = pool.tile(shape, f32, tag=f"{pref}_z", name=f"{pref}_z")
    nc.vector.tensor_mul(z[:], s[:], s[:])
    p = pool.tile(shape, f32, tag=f"{pref}_p", name=f"{pref}_p")
    nc.vector.tensor_scalar(
        p[:], z[:], 1.0 / 11.0, 1.0 / 9.0, op0=op.mult, op1=op.add
    )
    for c in (1.0 / 7.0, 1.0 / 5.0, 1.0 / 3.0):
        nc.vector.tensor_mul(p[:], p[:], z[:])
        nc.vector.tensor_scalar(p[:], p[:], c, None, op0=op.add)
    nc.vector.tensor_mul(p[:], p[:], z[:])  # Q = z*poly
    nc.vector.tensor_scalar(p[:], p[:], 1.0, None, op0=op.add)  # 1+Q
    nc.vector.tensor_mul(p[:], p[:], s[:])  # s*(1+Q)
    nc.vector.tensor_scalar(p[:], p[:], 2.0, None, op0=op.mult)  # ln(m)
    nc.vector.tensor_add(out_ap, p[:], et[:])


def _build_nc():
    import concourse.bass as bass  # noqa: F401
    import concourse.bass_isa as bass_isa
    import concourse.mybir as mybir
    import concourse.tile as tile
    from concourse import bacc

    f32 = mybir.dt.float32
    bf16 = mybir.dt.bfloat16
    AF = mybir.ActivationFunctionType
    op = mybir.AluOpType
    RED = bass_isa.ReduceOp
    AX = mybir.AxisListType
    RG = [list(range(N_CORES))]

    nc = bacc.Bacc(
        "TRN2", target_bir_lowering=False, debug=False, num_devices=N_CORES
    )

    # ---- kernel I/O ----
    xT = nc.dram_tensor("xT", [IN, B], bf16, kind="ExternalInput")
    w1s = nc.dram_tensor("w1s", [IN, SH], bf16, kind="ExternalInput")
    w2s = nc.dram_tensor("w2s", [H, SH], bf16, kind="ExternalInput")
    b1c = nc.dram_tensor("b1c", [128, KT], f32, kind="ExternalInput")
    b2f = nc.dram_tensor("b2f", [128, H // 128], f32, kind="ExternalInput")
    b2s = nc.dram_tensor("b2s", [SH], f32, kind="ExternalInput")
    w3s = nc.dram_tensor("w3s", [SH, OUT], f32, kind="ExternalInput")
    w3f = nc.dram_tensor("w3f", [128, (OUT * H) // 128], f32, kind="ExternalInput")
    rw3 = nc.dram_tensor("rw3", [128, (OUT * H) // 128], f32, kind="ExternalInput")
    b3r = nc.dram_tensor("b3r", [1, OUT], f32, kind="ExternalInput")
    rb3 = nc.dram_tensor("rb3", [1, OUT], f32, kind="ExternalInput")
    outB = nc.dram_tensor("outB", [B, OUT], f32, kind="ExternalOutput")
    scal = nc.dram_tensor("scal", [1, 8], f32, kind="ExternalOutput")

    with tile.TileContext(nc) as tc:
        with (
            tc.tile_pool(name="const", bufs=1) as const,
            tc.tile_pool(name="lnp", bufs=1) as lnp,
            tc.tile_pool(name="hst", bufs=4) as hstp,
            tc.tile_pool(name="acts", bufs=1) as acts,
            tc.tile_pool(name="psum", bufs=8, space="PSUM") as psum,
            tc.tile_pool(name="dram", bufs=1, space="DRAM") as dram,
        ):
            FR = (OUT * H) // 128  # 160

            # ---------- stream w1 + x (both bf16), then w2 ----------
            w1b = const.tile([128, KT * SH], bf16, name="w1b")
            xb = acts.tile([128, KT * B], bf16, tag="acts", name="xb")
            ss1c = const.tile([128, KT], f32, name="ss1c")
            sq_scr = const.tile([128, 512], f32, name="sq_scr")
            for t in range(KT):
                nc.sync.dma_start(
                    w1b[:, t * SH : (t + 1) * SH],
                    w1s.ap()[t * 128 : (t + 1) * 128, :],
                )
                nc.scalar.activation(
                    sq_scr[:],
                    w1b[:, t * SH : (t + 1) * SH],
                    AF.Square,
                    accum_out=ss1c[:, t : t + 1],
                )
                nc.sync.dma_start(
                    xb[:, t * B : (t + 1) * B],
                    xT.ap()[t * 128 : (t + 1) * 128, :],
                )

            w2b = const.tile([128, KT * SH], bf16, name="w2b")
            ss2c = const.tile([128, KT], f32, name="ss2c")
            for t in range(KT):
                nc.sync.dma_start(
                    w2b[:, t * SH : (t + 1) * SH],
                    w2s.ap()[t * 128 : (t + 1) * 128, :],
                )
                nc.scalar.activation(
                    sq_scr[:],
                    w2b[:, t * SH : (t + 1) * SH],
                    AF.Square,
                    accum_out=ss2c[:, t : t + 1],
                )

            # ---------- GEMM1 (t-outer: overlaps with the input DMA stream) ----------
            ps1 = [
                psum.tile([128, 512], f32, tag="ps", name=f"ps1_{g}")
                for g in range(OJ * NBB)
            ]
            for t in range(KT):
                for j in range(OJ):
                    for bb in range(NBB):
                        nc.tensor.matmul(
                            ps1[j * NBB + bb][:],
                            w1b[:, t * SH + j * 128 : t * SH + (j + 1) * 128],
                            xb[:, t * B + bb * 512 : t * B + (bb + 1) * 512],
                            start=(t == 0),
                            stop=(t == KT - 1),
                        )
            # evacuate unnormalized y1 as bf16 (normalize after the AllGather)
            y1sl = const.tile([128, OJ * B], bf16, name="y1sl")
            for j in range(OJ):
                for bb in range(NBB):
                    nc.vector.tensor_copy(
                        y1sl[:, j * B + bb * 512 : j * B + (bb + 1) * 512],
                        ps1[j * NBB + bb][:],
                    )

            # ---------- w-norm partials: one tiny AllGather + local sum ----------
            # (tiny AllReduces cost ~38us on this stack; a 64B AllGather is ~5us)
            ss1p = const.tile([128, 1], f32, name="ss1p")
            nc.vector.reduce_sum(ss1p[:], ss1c[:], axis=AX.X)
            ss1a = const.tile([128, 1], f32, name="ss1a")
            nc.gpsimd.partition_all_reduce(
                ss1a[:], ss1p[:], channels=128, reduce_op=RED.add
            )
            ss2p = const.tile([128, 1], f32, name="ss2p")
            nc.vector.reduce_sum(ss2p[:], ss2c[:], axis=AX.X)
            ss2a = const.tile([128, 1], f32, name="ss2a")
            nc.gpsimd.partition_all_reduce(
                ss2a[:], ss2p[:], channels=128, reduce_op=RED.add
            )
            ssag_in = dram.tile([1, 16], f32, name="ssag_in")
            ssag_out = dram.tile(
                [N_CORES, 16], f32, name="ssag_out", addr_space="Shared"
            )
            nc.vector.memset(sq_scr[:1, :16], 0.0)
            nc.sync.dma_start(ssag_in[:], sq_scr[:1, :16])
            nc.sync.dma_start(ssag_in[:, 0:1], ss1a[:1, :])
            nc.sync.dma_start(ssag_in[:, 1:2], ss2a[:1, :])
            nc.gpsimd.collective_compute(
                "AllGather",
                op.bypass,
                replica_groups=RG,
                ins=[ssag_in.opt()],
                outs=[ssag_out.opt()],
            )

            # ---------- AllGather of the unnormalized y1 shard (bf16) ----------
            ag1_in = dram.tile([SH, B], bf16, name="ag1_in")
            ag1_out = dram.tile(
                [N_CORES * SH, B], bf16, name="ag1_out", addr_space="Shared"
            )
            for j in range(OJ):
                nc.sync.dma_start(
                    ag1_in[j * 128 : (j + 1) * 128, :],
                    y1sl[:, j * B : (j + 1) * B],
                )
            nc.gpsimd.collective_compute(
                "AllGather",
                op.bypass,
                replica_groups=RG,
                ins=[ag1_in.opt()],
                outs=[ag1_out.opt()],
            )

            # ---------- small constants / scalar reductions ----------
            # (emitted after the heavy streams so they don't hog engines early;
            # all Exp ops precede every Sqrt so the ACT table switches just twice)
            rw3t = const.tile([128, FR], f32, name="rw3t")
            nc.sync.dma_start(rw3t[:], rw3.ap())
            nc.scalar.activation(rw3t[:], rw3t[:], AF.Exp)
            nc.vector.tensor_scalar(rw3t[:], rw3t[:], 1.0, None, op0=op.add)
            sw3 = const.tile([128, FR], f32, name="sw3")
            _emit_ln(nc, lnp, mybir, sw3[:], rw3t[:], [128, FR], "lnA")
            _emit_ln(nc, lnp, mybir, sw3[:], sw3[:], [128, FR], "lnB")
            slwp = const.tile([128, 1], f32, name="slwp")
            nc.vector.reduce_sum(slwp[:], sw3[:], axis=AX.X)
            slwa = const.tile([128, 1], f32, name="slwa")
            nc.gpsimd.partition_all_reduce(
                slwa[:], slwp[:], channels=128, reduce_op=RED.add
            )

            rb3t = const.tile([1, OUT], f32, name="rb3t")
            nc.sync.dma_start(rb3t[:], rb3.ap())
            nc.scalar.activation(rb3t[:], rb3t[:], AF.Exp)
            nc.vector.tensor_scalar(rb3t[:], rb3t[:], 1.0, None, op0=op.add)
            sb3 = const.tile([1, OUT], f32, name="sb3")
            _emit_ln(nc, lnp, mybir, sb3[:], rb3t[:], [1, OUT], "lnC")
            _emit_ln(nc, lnp, mybir, sb3[:], sb3[:], [1, OUT], "lnD")
            slb3 = const.tile([1, 1], f32, name="slb3")
            nc.vector.reduce_sum(slb3[:], sb3[:], axis=AX.X)

            # b1 / b2 norms (full vectors, locally on every core)
            b1t = const.tile([128, KT], f32, name="b1t")
            nc.sync.dma_start(b1t[:], b1c.ap())
            ssb1p = const.tile([128, 1], f32, name="ssb1p")
            nc.scalar.activation(
                sq_scr[:, :KT], b1t[:], AF.Square, accum_out=ssb1p[:]
            )
            ssb1a = const.tile([128, 1], f32, name="ssb1a")
            nc.gpsimd.partition_all_reduce(
                ssb1a[:], ssb1p[:], channels=128, reduce_op=RED.add
            )
            invb1 = const.tile([128, 1], f32, name="invb1")
            nc.vector.reciprocal(invb1[:], ssb1a[:])
            nc.scalar.sqrt(invb1[:], invb1[:])  # 1/||b1||
            mub1_all = const.tile([128, KT], f32, name="mub1_all")
            nc.vector.tensor_scalar(
                mub1_all[:], b1t[:], invb1[:, 0:1], None, op0=op.mult
            )

            b2t = const.tile([128, H // 128], f32, name="b2t")
            nc.sync.dma_start(b2t[:], b2f.ap())
            ssb2p = const.tile([128, 1], f32, name="ssb2p")
            nc.scalar.activation(
                sq_scr[:, : H // 128], b2t[:], AF.Square, accum_out=ssb2p[:]
            )
            ssb2a = const.tile([128, 1], f32, name="ssb2a")
            nc.gpsimd.partition_all_reduce(
                ssb2a[:], ssb2p[:], channels=128, reduce_op=RED.add
            )
            invb2 = const.tile([128, 1], f32, name="invb2")
            nc.vector.reciprocal(invb2[:], ssb2a[:])
            nc.scalar.sqrt(invb2[:], invb2[:])
            mub2 = []
            for j in range(OJ):
                t2 = const.tile([128, 1], f32, name=f"b2sl{j}")
                nc.sync.dma_start(t2[:], b2s.ap()[j * 128 : (j + 1) * 128][:, None])
                m2 = const.tile([128, 1], f32, name=f"mub2_{j}")
                nc.vector.tensor_mul(m2[:], t2[:], invb2[:])
                mub2.append(m2)

            # layer-3 weights (feature-sliced rows of w3^T) -> bf16
            w3b = []
            for j in range(OJ):
                t3 = const.tile([128, OUT], f32, name=f"w3st{j}")
                nc.sync.dma_start(t3[:], w3s.ap()[j * 128 : (j + 1) * 128, :])
                wb = const.tile([128, OUT], bf16, name=f"w3b{j}")
                nc.vector.tensor_copy(wb[:], t3[:])
                w3b.append(wb)

            # scalar reductions of layer-3 means (for the log-prior)
            w3t = const.tile([128, FR], f32, name="w3t")
            nc.sync.dma_start(w3t[:], w3f.ap())
            ssw3p = const.tile([128, 1], f32, name="ssw3p")
            nc.scalar.activation(
                sq_scr[:, :FR], w3t[:], AF.Square, accum_out=ssw3p[:]
            )
            ssw3a = const.tile([128, 1], f32, name="ssw3a")
            nc.gpsimd.partition_all_reduce(
                ssw3a[:], ssw3p[:], channels=128, reduce_op=RED.add
            )
            b3rt = const.tile([1, OUT], f32, name="b3rt")
            nc.sync.dma_start(b3rt[:], b3r.ap())
            ssb3 = const.tile([1, 1], f32, name="ssb3")
            nc.scalar.activation(
                sq_scr[:1, :OUT], b3rt[:], AF.Square, accum_out=ssb3[:]
            )

            # softmax bias pattern [1, NG*OUT] -> broadcast to 128 partitions
            bias40p = const.tile([1, NG * OUT], f32, name="bias40p")
            for g in range(NG):
                nc.vector.tensor_copy(bias40p[:, g * OUT : (g + 1) * OUT], b3rt[:])
            bias40 = const.tile([128, NG * OUT], f32, name="bias40")
            nc.gpsimd.partition_broadcast(bias40[:], bias40p[:], channels=128)

            # ---------- global norms from the gathered partials ----------
            sspr = const.tile([N_CORES, 2], f32, name="sspr")
            nc.sync.dma_start(sspr[:], ssag_out[:, 0:2])
            ssgs = const.tile([N_CORES, 2], f32, name="ssgs")
            nc.gpsimd.partition_all_reduce(
                ssgs[:], sspr[:], channels=N_CORES, reduce_op=RED.add
            )
            gss = const.tile([1, 2], f32, name="gss")
            nc.vector.tensor_copy(gss[:], ssgs[:1, :])
            invw = const.tile([1, 2], f32, name="invw")
            nc.vector.reciprocal(invw[:], gss[:])
            nc.scalar.sqrt(invw[:], invw[:])  # [1/||w1||, 1/||w2||]
            inv1b = const.tile([128, 1], f32, name="inv1b")
            nc.gpsimd.partition_broadcast(inv1b[:], invw[:, 0:1], channels=128)
            inv2b = const.tile([128, 1], f32, name="inv2b")
            nc.gpsimd.partition_broadcast(inv2b[:], invw[:, 1:2], channels=128)

            # h1 = relu(y1 * inv1 + mu_b1), full 4096 features on every core
            h1b = acts.tile([128, KT * B], bf16, tag="acts", name="h1b")
            for t in range(KT):
                hst = hstp.tile([128, B], bf16, tag="hst", name=f"hst{t}")
                nc.sync.dma_start(
                    hst[:], ag1_out[t * 128 : (t + 1) * 128, :]
                )
                nc.scalar.activation(
                    h1b[:, t * B : (t + 1) * B],
                    hst[:],
                    AF.Relu,
                    bias=mub1_all[:, t : t + 1],
                    scale=inv1b[:],
                )

            # ---------- GEMM2 ----------
            ps2 = [
                psum.tile([128, 512], f32, tag="ps", name=f"ps2_{g}")
                for g in range(OJ * NBB)
            ]
            for t in range(KT):
                for j in range(OJ):
                    for bb in range(NBB):
                        nc.tensor.matmul(
                            ps2[j * NBB + bb][:],
                            w2b[:, t * SH + j * 128 : t * SH + (j + 1) * 128],
                            h1b[:, t * B + bb * 512 : t * B + (bb + 1) * 512],
                            start=(t == 0),
                            stop=(t == KT - 1),
                        )
            h2sl = const.tile([128, OJ * B], bf16, name="h2sl")
            for j in range(OJ):
                for bb in range(NBB):
                    nc.scalar.activation(
                        h2sl[:, j * B + bb * 512 : j * B + (bb + 1) * 512],
                        ps2[j * NBB + bb][:],
                        AF.Relu,
                        bias=mub2[j][:],
                        scale=inv2b[:],
                    )

            # ---------- layer 3 (contraction-sharded) + AllReduce ----------
            ps3 = [
                psum.tile([OUT, 512], f32, tag="ps", name=f"ps3_{bb}")
                for bb in range(NBB)
            ]
            for j in range(OJ):
                for bb in range(NBB):
                    nc.tensor.matmul(
                        ps3[bb][:],
                        w3b[j][:],
                        h2sl[:, j * B + bb * 512 : j * B + (bb + 1) * 512],
                        start=(j == 0),
                        stop=(j == OJ - 1),
                    )
            ar3_in = dram.tile([OUT, B], f32, name="ar3_in")
            ar3_out = dram.tile([OUT, B], f32, name="ar3_out", addr_space="Shared")
            y3p = const.tile([OUT, B], f32, name="y3p")
            for bb in range(NBB):
                nc.vector.tensor_copy(
                    y3p[:, bb * 512 : (bb + 1) * 512], ps3[bb][:]
                )
            nc.sync.dma_start(ar3_in[:], y3p[:])
            nc.gpsimd.collective_compute(
                "AllReduce",
                op.add,
                replica_groups=RG,
                ins=[ar3_in.opt()],
                outs=[ar3_out.opt()],
            )

            # ---------- log-softmax, batch-major [128, NG, OUT] ----------
            y3r = const.tile([128, NG * OUT], f32, name="y3r")
            for g in range(NG):
                nc.sync.dma_start(
                    y3r[:, g * OUT : (g + 1) * OUT],
                    ar3_out[:, g * 128 : (g + 1) * 128].rearrange("c p -> p c"),
                )
            nc.vector.tensor_add(y3r[:], y3r[:], bias40[:])
            # |y3| is O(1) here, so exp() is safe without max-subtraction
            ex = const.tile([128, NG * OUT], f32, name="ex")
            nc.scalar.activation(ex[:], y3r[:], AF.Exp)
            sm = const.tile([128, NG], f32, name="sm")
            nc.vector.reduce_sum(
                sm[:], ex[:].rearrange("p (g c) -> p g c", g=NG), axis=AX.X
            )
            ls = const.tile([128, NG], f32, name="ls")
            _emit_ln(nc, lnp, mybir, ls[:], sm[:], [128, NG], "lnS")
            for g in range(NG):
                nc.vector.tensor_scalar(
                    y3r[:, g * OUT : (g + 1) * OUT],
                    y3r[:, g * OUT : (g + 1) * OUT],
                    ls[:, g : g + 1],
                    None,
                    op0=op.subtract,
                )
            nc.sync.dma_start(
                outB.ap().rearrange("(g p) c -> p g c", p=128),
                y3r[:].rearrange("p (g c) -> p g c", g=NG),
            )

            # ---------- scalar outputs ----------
            scal_sb = const.tile([1, 8], f32, name="scal_sb")
            nc.vector.tensor_copy(scal_sb[:, 0:1], gss[:, 0:1])
            nc.vector.tensor_copy(scal_sb[:, 1:2], gss[:, 1:2])
            nc.vector.tensor_copy(scal_sb[:, 2:3], ssb1a[:1, :])
            nc.vector.tensor_copy(scal_sb[:, 3:4], ssb2a[:1, :])
            nc.vector.tensor_copy(scal_sb[:, 4:5], slwa[:1, :])
            nc.vector.tensor_copy(scal_sb[:, 5:6], slb3[:])
            nc.vector.tensor_copy(scal_sb[:, 6:7], ssw3a[:1, :])
            nc.vector.tensor_copy(scal_sb[:, 7:8], ssb3[:])
            nc.sync.dma_start(scal.ap(), scal_sb[:])

    nc.compile()
    return nc


def _log_surface_area(d):
    h = (d + 1.0) / 2.0
    return math.log(2.0) + h * math.log(math.pi) - math.lgamma(h)


def _log_besseli(s, kappa):
    x = kappa / s
    sq = math.sqrt(1.0 + x * x)
    eta = sq + math.log(x) - math.log1p(sq)
    return s * eta - 0.5 * math.log(2.0 * math.pi * s) - 0.5 * math.log(sq)


def _log_C_vmf(d, kappa):
    s = 0.5 * d - 1.0
    return d * NHLOG2PI + s * math.log(kappa) - _log_besseli(s, kappa)


LAST_RESULTS = None


def kernel(
    x,
    w1_mu,
    w1_logkappa,
    b1_mu,
    b1_logkappa,
    w2_mu,
    w2_logkappa,
    b2_mu,
    b2_logkappa,
    w3_mu,
    w3_rho,
    b3_mu,
    b3_rho,
):
    global LAST_RESULTS
    import ml_dtypes

    from concourse import bass_utils

    if "nc" not in _CACHE:
        _CACHE["nc"] = _build_nc()
    nc = _CACHE["nc"]

    f = np.float32
    x = np.asarray(x, f)
    W1 = np.asarray(w1_mu, f).reshape(IN, H).astype(ml_dtypes.bfloat16)
    W2 = np.asarray(w2_mu, f).reshape(H, H).astype(ml_dtypes.bfloat16)
    b1 = np.ascontiguousarray(np.asarray(b1_mu, f))
    b2 = np.ascontiguousarray(np.asarray(b2_mu, f))
    w3 = np.asarray(w3_mu, f)
    w3T = np.ascontiguousarray(w3.T)
    b3 = np.ascontiguousarray(np.asarray(b3_mu, f))
    xTc = np.ascontiguousarray(x.T).astype(ml_dtypes.bfloat16)
    b1cols = np.ascontiguousarray(b1.reshape(KT, 128).T)
    w3flat = np.ascontiguousarray(w3.reshape(128, (OUT * H) // 128))
    rw3m = np.ascontiguousarray(np.asarray(w3_rho, f).reshape(128, (OUT * H) // 128))
    b3row = b3.reshape(1, OUT)
    rb3m = np.ascontiguousarray(np.asarray(b3_rho, f).reshape(1, OUT))

    in_maps = []
    for c in range(N_CORES):
        sl = slice(c * SH, (c + 1) * SH)
        in_maps.append(
            {
                "xT": xTc,
                "w1s": np.ascontiguousarray(W1[:, sl]),
                "w2s": np.ascontiguousarray(W2[:, sl]),
                "b1c": b1cols,
                "b2f": b2.reshape(128, H // 128),
                "b2s": np.ascontiguousarray(b2[sl]),
                "w3s": np.ascontiguousarray(w3T[sl, :]),
                "w3f": w3flat,
                "rw3": rw3m,
                "b3r": b3row,
                "rb3": rb3m,
            }
        )

    res = bass_utils.run_bass_kernel_spmd(nc, in_maps, core_ids=list(range(N_CORES)))
    LAST_RESULTS = res
    r0 = res.results[0]
    out = np.ascontiguousarray(r0["outB"].astype(np.float32))
    s = r0["scal"][0].astype(np.float64)

    kw1 = math.exp(float(np.float32(w1_logkappa))) + 1e-6
    kb1 = math.exp(float(np.float32(b1_logkappa))) + 1e-6
    kw2 = math.exp(float(np.float32(w2_logkappa))) + 1e-6
    kb2 = math.exp(float(np.float32(b2_logkappa))) + 1e-6
    d_w = float(H * IN)
    d_b = float(H)
    n3 = OUT * H + OUT  # 20485 gaussian params

    lvp = (
        kw1
        + _log_C_vmf(d_w, kw1)
        + kb1
        + _log_C_vmf(d_b, kb1)
        + kw2
        + _log_C_vmf(d_w, kw2)
        + kb2
        + _log_C_vmf(d_b, kb2)
        + n3 * NHLOG2PI
        - (s[4] + s[5])
    )
    lp = -4.0 * _log_surface_area(d_w) + n3 * NHLOG2PI - 0.5 * (s[6] + s[7])

    return out, np.asarray(lvp, np.float32), np.asarray(lp, np.float32)


# revision 21
# speedup vs baseline: 1.1434x; 1.1185x over previous
"""Bayesian MLP forward on 8 Trainium2 NeuronCores.

Strategy: 8-way model parallelism over the hidden dimension (512 output
features per core), feature-major activations [feat, batch] so no
transposes are needed on device. bf16 matmuls with f32 PSUM
accumulation.

Cross-core exchange is ONE fused AllGather: each core contributes its
unnormalized y1 shard (bf16) plus its w1/w2 sum-of-squares partials
packed into an extra row; every core then reduces the partials locally,
forms the vMF weight norms, and applies scale+bias+relu to the full
gathered h1. Layer 3 is sharded over the contraction dim and finished
with a 20KB AllReduce, then log-softmax over the 5 classes is computed
batch-major ([128 x 8 x 5]) so nothing runs partition-starved. TRN2's
ACT engine has no Ln table, so ln() is evaluated on the vector engine
from exponent/mantissa bits plus an atanh-series polynomial.
"""

import math

import numpy as np

N_CORES = 8
B, IN, H, OUT = 1024, 4096, 4096, 5
SH = H // N_CORES  # 512 features per core
KT = IN // 128  # 32 contraction tiles (same for layer 2)
OJ = SH // 128  # 4 output-feature tiles per core
NBB = B // 512  # 2 batch halves of 512
NG = B // 128  # 8 batch groups for the softmax tail
AGR = SH + 1  # rows per rank in the fused AllGather (512 data + 1 meta)

NHLOG2PI = -0.5 * math.log(2.0 * math.pi)
LN2 = math.log(2.0)

_CACHE = {}


def _emit_ln(nc, pool, mybir, out_ap, in_ap, shape, pref):
    """out = ln(in) elementwise for positive f32 input, on the DVE.

    x = 2^e * m, m in [1,2): ln x = e*ln2 + 2*atanh((m-1)/(m+1)) with the
    atanh series in z = s^2 truncated at z^5 (|s| <= 1/3 so the
    truncation error is ~2.5e-8).
    """
    f32 = mybir.dt.float32
    u32 = mybir.dt.uint32
    op = mybir.AluOpType
    xi = in_ap.bitcast(u32)

    eu = pool.tile(shape, u32, tag=f"{pref}_eu", name=f"{pref}_eu")
    nc.vector.tensor_scalar(eu[:], xi, 23, None, op0=op.logical_shift_right)
    ef = pool.tile(shape, f32, tag=f"{pref}_ef", name=f"{pref}_ef")
    nc.vector.tensor_copy(ef[:], eu[:])
    et = pool.tile(shape, f32, tag=f"{pref}_et", name=f"{pref}_et")
    nc.vector.tensor_scalar(
        et[:], ef[:], LN2, 127.0 * LN2, op0=op.mult, op1=op.subtract
    )

    mu = pool.tile(shape, u32, tag=f"{pref}_mu", name=f"{pref}_mu")
    nc.vector.tensor_scalar(
        mu[:], xi, 0x007FFFFF, 0x3F800000, op0=op.bitwise_and, op1=op.bitwise_or
    )
    m = mu[:].bitcast(f32)
    num = pool.tile(shape, f32, tag=f"{pref}_num", name=f"{pref}_num")
    nc.vector.tensor_scalar(num[:], m, 1.0, None, op0=op.subtract)
    den = pool.tile(shape, f32, tag=f"{pref}_den", name=f"{pref}_den")
    nc.vector.tensor_scalar(den[:], m, 1.0, None, op0=op.add)
    nc.vector.reciprocal(den[:], den[:])
    s = pool.tile(shape, f32, tag=f"{pref}_s", name=f"{pref}_s")
    nc.vector.tensor_mul(s[:], num[:], den[:])
    z = pool.tile(shape, f32, tag=f"{pref}_z", name=f"{pref}_z")
    nc.vector.tensor_mul(z[:], s[:], s[:])
    p = pool.tile(shape, f32, tag=f"{pref}_p", name=f"{pref}_p")
    nc.vector.tensor_scalar(
        p[:], z[:], 1.0 / 11.0, 1.0 / 9.0, op0=op.mult, op1=op.add
    )
    for c in (1.0 / 7.0, 1.0 / 5.0, 1.0 / 3.0):
        nc.vector.tensor_mul(p[:], p[:], z[:])
        nc.vector.tensor_scalar(p[:], p[:], c, None, op0=op.add)
    nc.vector.tensor_mul(p[:], p[:], z[:])  # Q = z*poly
    nc.vector.tensor_scalar(p[:], p[:], 1.0, None, op0=op.add)  # 1+Q
    nc.vector.tensor_mul(p[:], p[:], s[:])  # s*(1+Q)
    nc.vector.tensor_scalar(p[:], p[:], 2.0, None, op0=op.mult)  # ln(m)
    nc.vector.tensor_add(out_ap, p[:], et[:])


def _build_nc():
    import concourse.bass as bass  # noqa: F401
    import concourse.bass_isa as bass_isa
    import concourse.mybir as mybir
    import concourse.tile as tile
    from concourse import bacc

    f32 = mybir.dt.float32
    bf16 = mybir.dt.bfloat16
    AF = mybir.ActivationFunctionType
    op = mybir.AluOpType
    RED = bass_isa.ReduceOp
    AX = mybir.AxisListType
    RG = [list(range(N_CORES))]

    nc = bacc.Bacc(
        "TRN2", target_bir_lowering=False, debug=False, num_devices=N_CORES
    )

    # ---- kernel I/O ----
    xT = nc.dram_tensor("xT", [IN, B], bf16, kind="ExternalInput")
    w1s = nc.dram_tensor("w1s", [IN, SH], bf16, kind="ExternalInput")
    w2s = nc.dram_tensor("w2s", [H, SH], bf16, kind="ExternalInput")
    b1c = nc.dram_tensor("b1c", [128, KT], f32, kind="ExternalInput")
    b2f = nc.dram_tensor("b2f", [128, H // 128], f32, kind="ExternalInput")
    b2s = nc.dram_tensor("b2s", [SH], f32, kind="ExternalInput")
    w3s = nc.dram_tensor("w3s", [SH, OUT], f32, kind="ExternalInput")
    w3f = nc.dram_tensor("w3f", [128, (OUT * H) // 128], f32, kind="ExternalInput")
    rw3 = nc.dram_tensor("rw3", [128, (OUT * H) // 128], f32, kind="ExternalInput")
    b3r = nc.dram_tensor("b3r", [1, OUT], f32, kind="ExternalInput")
    rb3 = nc.dram_tensor("rb3", [1, OUT], f32, kind="ExternalInput")
    outB = nc.dram_tensor("outB", [B, OUT], f32, kind="ExternalOutput")
    scal = nc.dram_tensor("scal", [1, 8], f32, kind="ExternalOutput")

    with tile.TileContext(nc) as tc:
        with (
            tc.tile_pool(name="const", bufs=1) as const,
            tc.tile_pool(name="lnp", bufs=1) as lnp,
            tc.tile_pool(name="hst", bufs=4) as hstp,
            tc.tile_pool(name="acts", bufs=1) as acts,
            tc.tile_pool(name="psum", bufs=8, space="PSUM") as psum,
            tc.tile_pool(name="dram", bufs=1, space="DRAM") as dram,
        ):
            FR = (OUT * H) // 128  # 160

            # ---------- stream w1 + x (both bf16), then w2 ----------
            w1b = const.tile([128, KT * SH], bf16, name="w1b")
            xb = acts.tile([128, KT * B], bf16, tag="acts", name="xb")
            ss1c = const.tile([128, KT], f32, name="ss1c")
            sq_scr = const.tile([128, 512], f32, name="sq_scr")
            for t in range(KT):
                nc.sync.dma_start(
                    w1b[:, t * SH : (t + 1) * SH],
                    w1s.ap()[t * 128 : (t + 1) * 128, :],
                )
                nc.scalar.activation(
                    sq_scr[:],
                    w1b[:, t * SH : (t + 1) * SH],
                    AF.Square,
                    accum_out=ss1c[:, t : t + 1],
                )
                nc.sync.dma_start(
                    xb[:, t * B : (t + 1) * B],
                    xT.ap()[t * 128 : (t + 1) * 128, :],
                )

            w2b = const.tile([128, KT * SH], bf16, name="w2b")
            ss2c = const.tile([128, KT], f32, name="ss2c")
            for t in range(KT):
                nc.sync.dma_start(
                    w2b[:, t * SH : (t + 1) * SH],
                    w2s.ap()[t * 128 : (t + 1) * 128, :],
                )
                nc.scalar.activation(
                    sq_scr[:],
                    w2b[:, t * SH : (t + 1) * SH],
                    AF.Square,
                    accum_out=ss2c[:, t : t + 1],
                )

            # ---------- GEMM1 (t-outer: overlaps with the input DMA stream) ----------
            ps1 = [
                psum.tile([128, 512], f32, tag="ps", name=f"ps1_{g}")
                for g in range(OJ * NBB)
            ]
            for t in range(KT):
                for j in range(OJ):
                    for bb in range(NBB):
                        nc.tensor.matmul(
                            ps1[j * NBB + bb][:],
                            w1b[:, t * SH + j * 128 : t * SH + (j + 1) * 128],
                            xb[:, t * B + bb * 512 : t * B + (bb + 1) * 512],
                            start=(t == 0),
                            stop=(t == KT - 1),
                        )
            # evacuate unnormalized y1 as bf16 (normalize after the AllGather)
            y1sl = const.tile([128, OJ * B], bf16, name="y1sl")
            for j in range(OJ):
                for bb in range(NBB):
                    nc.vector.tensor_copy(
                        y1sl[:, j * B + bb * 512 : j * B + (bb + 1) * 512],
                        ps1[j * NBB + bb][:],
                    )

            # ---------- ||w1||^2 partials: tiny AllGather + local sum ----------
            # The first real collective pays a ~45us init on this stack, so it
            # must be one whose input is ready early (ss1, ~60us) — it then
            # absorbs the init while GEMM1 still runs; later collectives are
            # fast (~10us). ss2 gets its own (fast) AllGather later.
            ss1p = const.tile([128, 1], f32, name="ss1p")
            nc.vector.reduce_sum(ss1p[:], ss1c[:], axis=AX.X)
            ss1a = const.tile([128, 1], f32, name="ss1a")
            nc.gpsimd.partition_all_reduce(
                ss1a[:], ss1p[:], channels=128, reduce_op=RED.add
            )
            ssag_in = dram.tile([1, 16], f32, name="ssag_in")
            ssag_out = dram.tile(
                [N_CORES, 16], f32, name="ssag_out", addr_space="Shared"
            )
            nc.gpsimd.dma_start(ssag_in[:, 0:1], ss1a[:1, :])
            nc.gpsimd.collective_compute(
                "AllGather",
                op.bypass,
                replica_groups=RG,
                ins=[ssag_in.opt()],
                outs=[ssag_out.opt()],
            )

            # ---------- AllGather of the unnormalized y1 shard (bf16) ----------
            ag1_in = dram.tile([SH, B], bf16, name="ag1_in")
            ag1_out = dram.tile(
                [N_CORES * SH, B], bf16, name="ag1_out", addr_space="Shared"
            )
            for j in range(OJ):
                nc.gpsimd.dma_start(
                    ag1_in[j * 128 : (j + 1) * 128, :],
                    y1sl[:, j * B : (j + 1) * B],
                )
            nc.gpsimd.collective_compute(
                "AllGather",
                op.bypass,
                replica_groups=RG,
                ins=[ag1_in.opt()],
                outs=[ag1_out.opt()],
            )

            # ---------- ||w2||^2 partials (fast third collective, hidden) ----------
            ss2p = const.tile([128, 1], f32, name="ss2p")
            nc.vector.reduce_sum(ss2p[:], ss2c[:], axis=AX.X)
            ss2a = const.tile([128, 1], f32, name="ss2a")
            nc.gpsimd.partition_all_reduce(
                ss2a[:], ss2p[:], channels=128, reduce_op=RED.add
            )
            ssag2_in = dram.tile([1, 16], f32, name="ssag2_in")
            ssag2_out = dram.tile(
                [N_CORES, 16], f32, name="ssag2_out", addr_space="Shared"
            )
            nc.gpsimd.dma_start(ssag2_in[:, 0:1], ss2a[:1, :])
            nc.gpsimd.collective_compute(
                "AllGather",
                op.bypass,
                replica_groups=RG,
                ins=[ssag2_in.opt()],
                outs=[ssag2_out.opt()],
            )

            # ---------- small constants / scalar reductions ----------
            # (emitted after the heavy streams so they don't hog engines early;
            # all Exp ops precede every Sqrt so the ACT table switches just twice)
            rw3t = const.tile([128, FR], f32, name="rw3t")
            nc.sync.dma_start(rw3t[:], rw3.ap())
            nc.scalar.activation(rw3t[:], rw3t[:], AF.Exp)
            nc.vector.tensor_scalar(rw3t[:], rw3t[:], 1.0, None, op0=op.add)
            sw3 = const.tile([128, FR], f32, name="sw3")
            _emit_ln(nc, lnp, mybir, sw3[:], rw3t[:], [128, FR], "lnA")
            _emit_ln(nc, lnp, mybir, sw3[:], sw3[:], [128, FR], "lnB")
            slwp = const.tile([128, 1], f32, name="slwp")
            nc.vector.reduce_sum(slwp[:], sw3[:], axis=AX.X)
            slwa = const.tile([128, 1], f32, name="slwa")
            nc.gpsimd.partition_all_reduce(
                slwa[:], slwp[:], channels=128, reduce_op=RED.add
            )

            rb3t = const.tile([1, OUT], f32, name="rb3t")
            nc.sync.dma_start(rb3t[:], rb3.ap())
            nc.scalar.activation(rb3t[:], rb3t[:], AF.Exp)
            nc.vector.tensor_scalar(rb3t[:], rb3t[:], 1.0, None, op0=op.add)
            sb3 = const.tile([1, OUT], f32, name="sb3")
            _emit_ln(nc, lnp, mybir, sb3[:], rb3t[:], [1, OUT], "lnC")
            _emit_ln(nc, lnp, mybir, sb3[:], sb3[:], [1, OUT], "lnD")
            slb3 = const.tile([1, 1], f32, name="slb3")
            nc.vector.reduce_sum(slb3[:], sb3[:], axis=AX.X)

            # b1 / b2 norms (full vectors, locally on every core)
            b1t = const.tile([128, KT], f32, name="b1t")
            nc.sync.dma_start(b1t[:], b1c.ap())
            ssb1p = const.tile([128, 1], f32, name="ssb1p")
            nc.scalar.activation(
                sq_scr[:, :KT], b1t[:], AF.Square, accum_out=ssb1p[:]
            )
            ssb1a = const.tile([128, 1], f32, name="ssb1a")
            nc.gpsimd.partition_all_reduce(
                ssb1a[:], ssb1p[:], channels=128, reduce_op=RED.add
            )
            invb1 = const.tile([128, 1], f32, name="invb1")
            nc.vector.reciprocal(invb1[:], ssb1a[:])
            nc.scalar.sqrt(invb1[:], invb1[:])  # 1/||b1||
            mub1_all = const.tile([128, KT], f32, name="mub1_all")
            nc.vector.tensor_scalar(
                mub1_all[:], b1t[:], invb1[:, 0:1], None, op0=op.mult
            )

            b2t = const.tile([128, H // 128], f32, name="b2t")
            nc.sync.dma_start(b2t[:], b2f.ap())
            ssb2p = const.tile([128, 1], f32, name="ssb2p")
            nc.scalar.activation(
                sq_scr[:, : H // 128], b2t[:], AF.Square, accum_out=ssb2p[:]
            )
            ssb2a = const.tile([128, 1], f32, name="ssb2a")
            nc.gpsimd.partition_all_reduce(
                ssb2a[:], ssb2p[:], channels=128, reduce_op=RED.add
            )
            invb2 = const.tile([128, 1], f32, name="invb2")
            nc.vector.reciprocal(invb2[:], ssb2a[:])
            nc.scalar.sqrt(invb2[:], invb2[:])
            mub2 = []
            for j in range(OJ):
                t2 = const.tile([128, 1], f32, name=f"b2sl{j}")
                nc.sync.dma_start(t2[:], b2s.ap()[j * 128 : (j + 1) * 128][:, None])
                m2 = const.tile([128, 1], f32, name=f"mub2_{j}")
                nc.vector.tensor_mul(m2[:], t2[:], invb2[:])
                mub2.append(m2)

            # layer-3 weights (feature-sliced rows of w3^T) -> bf16
            w3b = []
            for j in range(OJ):
                t3 = const.tile([128, OUT], f32, name=f"w3st{j}")
                nc.sync.dma_start(t3[:], w3s.ap()[j * 128 : (j + 1) * 128, :])
                wb = const.tile([128, OUT], bf16, name=f"w3b{j}")
                nc.vector.tensor_copy(wb[:], t3[:])
                w3b.append(wb)

            # scalar reductions of layer-3 means (for the log-prior)
            w3t = const.tile([128, FR], f32, name="w3t")
            nc.sync.dma_start(w3t[:], w3f.ap())
            ssw3p = const.tile([128, 1], f32, name="ssw3p")
            nc.scalar.activation(
                sq_scr[:, :FR], w3t[:], AF.Square, accum_out=ssw3p[:]
            )
            ssw3a = const.tile([128, 1], f32, name="ssw3a")
            nc.gpsimd.partition_all_reduce(
                ssw3a[:], ssw3p[:], channels=128, reduce_op=RED.add
            )
            b3rt = const.tile([1, OUT], f32, name="b3rt")
            nc.sync.dma_start(b3rt[:], b3r.ap())
            ssb3 = const.tile([1, 1], f32, name="ssb3")
            nc.scalar.activation(
                sq_scr[:1, :OUT], b3rt[:], AF.Square, accum_out=ssb3[:]
            )

            # softmax bias pattern [1, NG*OUT] -> broadcast to 128 partitions
            bias40p = const.tile([1, NG * OUT], f32, name="bias40p")
            for g in range(NG):
                nc.vector.tensor_copy(bias40p[:, g * OUT : (g + 1) * OUT], b3rt[:])
            bias40 = const.tile([128, NG * OUT], f32, name="bias40")
            nc.gpsimd.partition_broadcast(bias40[:], bias40p[:], channels=128)

            # ---------- global norms from the gathered partials ----------
            sspr = const.tile([N_CORES, 1], f32, name="sspr")
            nc.sync.dma_start(sspr[:], ssag_out[:, 0:1])
            ssgs = const.tile([N_CORES, 1], f32, name="ssgs")
            nc.gpsimd.partition_all_reduce(
                ssgs[:], sspr[:], channels=N_CORES, reduce_op=RED.add
            )
            gs1 = const.tile([1, 1], f32, name="gs1")
            nc.vector.tensor_copy(gs1[:], ssgs[:1, :])
            inv1 = const.tile([1, 1], f32, name="inv1")
            nc.vector.reciprocal(inv1[:], gs1[:])
            nc.scalar.sqrt(inv1[:], inv1[:])  # 1/||w1||
            inv1b = const.tile([128, 1], f32, name="inv1b")
            nc.gpsimd.partition_broadcast(inv1b[:], inv1[:], channels=128)

            sspr2 = const.tile([N_CORES, 1], f32, name="sspr2")
            nc.sync.dma_start(sspr2[:], ssag2_out[:, 0:1])
            ssgs2 = const.tile([N_CORES, 1], f32, name="ssgs2")
            nc.gpsimd.partition_all_reduce(
                ssgs2[:], sspr2[:], channels=N_CORES, reduce_op=RED.add
            )
            gs2 = const.tile([1, 1], f32, name="gs2")
            nc.vector.tensor_copy(gs2[:], ssgs2[:1, :])
            inv2 = const.tile([1, 1], f32, name="inv2")
            nc.vector.reciprocal(inv2[:], gs2[:])
            nc.scalar.sqrt(inv2[:], inv2[:])  # 1/||w2||
            inv2b = const.tile([128, 1], f32, name="inv2b")
            nc.gpsimd.partition_broadcast(inv2b[:], inv2[:], channels=128)

            # h1 = relu(y1 * inv1 + mu_b1), full 4096 features on every core
            h1b = acts.tile([128, KT * B], bf16, tag="acts", name="h1b")
            for t in range(KT):
                hst = hstp.tile([128, B], bf16, tag="hst", name=f"hst{t}")
                nc.sync.dma_start(
                    hst[:], ag1_out[t * 128 : (t + 1) * 128, :]
                )
                nc.scalar.activation(
                    h1b[:, t * B : (t + 1) * B],
                    hst[:],
                    AF.Relu,
                    bias=mub1_all[:, t : t + 1],
                    scale=inv1b[:],
                )

            # ---------- GEMM2 ----------
            ps2 = [
                psum.tile([128, 512], f32, tag="ps", name=f"ps2_{g}")
                for g in range(OJ * NBB)
            ]
            for t in range(KT):
                for j in range(OJ):
                    for bb in range(NBB):
                        nc.tensor.matmul(
                            ps2[j * NBB + bb][:],
                            w2b[:, t * SH + j * 128 : t * SH + (j + 1) * 128],
                            h1b[:, t * B + bb * 512 : t * B + (bb + 1) * 512],
                            start=(t == 0),
                            stop=(t == KT - 1),
                        )
            h2sl = const.tile([128, OJ * B], bf16, name="h2sl")
            for j in range(OJ):
                for bb in range(NBB):
                    nc.scalar.activation(
                        h2sl[:, j * B + bb * 512 : j * B + (bb + 1) * 512],
                        ps2[j * NBB + bb][:],
                        AF.Relu,
                        bias=mub2[j][:],
                        scale=inv2b[:],
                    )

            # ---------- layer 3 (contraction-sharded) + AllReduce ----------
            ps3 = [
                psum.tile([OUT, 512], f32, tag="ps", name=f"ps3_{bb}")
                for bb in range(NBB)
            ]
            for j in range(OJ):
                for bb in range(NBB):
                    nc.tensor.matmul(
                        ps3[bb][:],
                        w3b[j][:],
                        h2sl[:, j * B + bb * 512 : j * B + (bb + 1) * 512],
                        start=(j == 0),
                        stop=(j == OJ - 1),
                    )
            # AllReduce buffers laid out [(g c), p] so both the store and the
            # batch-major reload are single 3-dim DMAs.
            ar3_in = dram.tile([NG * OUT, 128], f32, name="ar3_in")
            ar3_out = dram.tile(
                [NG * OUT, 128], f32, name="ar3_out", addr_space="Shared"
            )
            y3p = const.tile([OUT, B], f32, name="y3p")
            for bb in range(NBB):
                nc.vector.tensor_copy(
                    y3p[:, bb * 512 : (bb + 1) * 512], ps3[bb][:]
                )
            nc.sync.dma_start(
                ar3_in.rearrange("(g c) p -> c g p", c=OUT),
                y3p[:].rearrange("c (g p) -> c g p", p=128),
            )
            nc.gpsimd.collective_compute(
                "AllReduce",
                op.add,
                replica_groups=RG,
                ins=[ar3_in.opt()],
                outs=[ar3_out.opt()],
            )

            # ---------- log-softmax, batch-major [128, NG, OUT] ----------
            y3r = const.tile([128, NG * OUT], f32, name="y3r")
            nc.sync.dma_start(
                y3r[:].rearrange("p (g c) -> p g c", g=NG),
                ar3_out.rearrange("(g c) p -> p g c", c=OUT),
            )
            nc.vector.tensor_add(y3r[:], y3r[:], bias40[:])
            # |y3| is O(1) here, so exp() is safe without max-subtraction
            ex = const.tile([128, NG * OUT], f32, name="ex")
            nc.scalar.activation(ex[:], y3r[:], AF.Exp)
            sm = const.tile([128, NG], f32, name="sm")
            nc.vector.reduce_sum(
                sm[:], ex[:].rearrange("p (g c) -> p g c", g=NG), axis=AX.X
            )
            ls = const.tile([128, NG], f32, name="ls")
            _emit_ln(nc, lnp, mybir, ls[:], sm[:], [128, NG], "lnS")
            nc.vector.tensor_sub(
                y3r[:].rearrange("p (g c) -> p g c", g=NG),
                y3r[:].rearrange("p (g c) -> p g c", g=NG),
                ls[:].unsqueeze(-1).broadcast_to([128, NG, OUT]),
            )
            nc.sync.dma_start(
                outB.ap().rearrange("(g p) c -> p g c", p=128),
                y3r[:].rearrange("p (g c) -> p g c", g=NG),
            )

            # ---------- scalar outputs ----------
            scal_sb = const.tile([1, 8], f32, name="scal_sb")
            nc.vector.tensor_copy(scal_sb[:, 0:1], gs1[:])
            nc.vector.tensor_copy(scal_sb[:, 1:2], gs2[:])
            nc.vector.tensor_copy(scal_sb[:, 2:3], ssb1a[:1, :])
            nc.vector.tensor_copy(scal_sb[:, 3:4], ssb2a[:1, :])
            nc.vector.tensor_copy(scal_sb[:, 4:5], slwa[:1, :])
            nc.vector.tensor_copy(scal_sb[:, 5:6], slb3[:])
            nc.vector.tensor_copy(scal_sb[:, 6:7], ssw3a[:1, :])
            nc.vector.tensor_copy(scal_sb[:, 7:8], ssb3[:])
            nc.sync.dma_start(scal.ap(), scal_sb[:])

    nc.compile()
    return nc


def _log_surface_area(d):
    h = (d + 1.0) / 2.0
    return math.log(2.0) + h * math.log(math.pi) - math.lgamma(h)


def _log_besseli(s, kappa):
    x = kappa / s
    sq = math.sqrt(1.0 + x * x)
    eta = sq + math.log(x) - math.log1p(sq)
    return s * eta - 0.5 * math.log(2.0 * math.pi * s) - 0.5 * math.log(sq)


def _log_C_vmf(d, kappa):
    s = 0.5 * d - 1.0
    return d * NHLOG2PI + s * math.log(kappa) - _log_besseli(s, kappa)


LAST_RESULTS = None


def kernel(
    x,
    w1_mu,
    w1_logkappa,
    b1_mu,
    b1_logkappa,
    w2_mu,
    w2_logkappa,
    b2_mu,
    b2_logkappa,
    w3_mu,
    w3_rho,
    b3_mu,
    b3_rho,
):
    global LAST_RESULTS
    import ml_dtypes

    from concourse import bass_utils

    if "nc" not in _CACHE:
        _CACHE["nc"] = _build_nc()
    nc = _CACHE["nc"]

    f = np.float32
    x = np.asarray(x, f)
    W1 = np.asarray(w1_mu, f).reshape(IN, H).astype(ml_dtypes.bfloat16)
    W2 = np.asarray(w2_mu, f).reshape(H, H).astype(ml_dtypes.bfloat16)
    b1 = np.ascontiguousarray(np.asarray(b1_mu, f))
    b2 = np.ascontiguousarray(np.asarray(b2_mu, f))
    w3 = np.asarray(w3_mu, f)
    w3T = np.ascontiguousarray(w3.T)
    b3 = np.ascontiguousarray(np.asarray(b3_mu, f))
    xTc = np.ascontiguousarray(x.T).astype(ml_dtypes.bfloat16)
    b1cols = np.ascontiguousarray(b1.reshape(KT, 128).T)
    w3flat = np.ascontiguousarray(w3.reshape(128, (OUT * H) // 128))
    rw3m = np.ascontiguousarray(np.asarray(w3_rho, f).reshape(128, (OUT * H) // 128))
    b3row = b3.reshape(1, OUT)
    rb3m = np.ascontiguousarray(np.asarray(b3_rho, f).reshape(1, OUT))

    in_maps = []
    for c in range(N_CORES):
        sl = slice(c * SH, (c + 1) * SH)
        in_maps.append(
            {
                "xT": xTc,
                "w1s": np.ascontiguousarray(W1[:, sl]),
                "w2s": np.ascontiguousarray(W2[:, sl]),
                "b1c": b1cols,
                "b2f": b2.reshape(128, H // 128),
                "b2s": np.ascontiguousarray(b2[sl]),
                "w3s": np.ascontiguousarray(w3T[sl, :]),
                "w3f": w3flat,
                "rw3": rw3m,
                "b3r": b3row,
                "rb3": rb3m,
            }
        )

    res = bass_utils.run_bass_kernel_spmd(nc, in_maps, core_ids=list(range(N_CORES)))
    LAST_RESULTS = res
    r0 = res.results[0]
    out = np.ascontiguousarray(r0["outB"].astype(np.float32))
    s = r0["scal"][0].astype(np.float64)

    kw1 = math.exp(float(np.float32(w1_logkappa))) + 1e-6
    kb1 = math.exp(float(np.float32(b1_logkappa))) + 1e-6
    kw2 = math.exp(float(np.float32(w2_logkappa))) + 1e-6
    kb2 = math.exp(float(np.float32(b2_logkappa))) + 1e-6
    d_w = float(H * IN)
    d_b = float(H)
    n3 = OUT * H + OUT  # 20485 gaussian params

    lvp = (
        kw1
        + _log_C_vmf(d_w, kw1)
        + kb1
        + _log_C_vmf(d_b, kb1)
        + kw2
        + _log_C_vmf(d_w, kw2)
        + kb2
        + _log_C_vmf(d_b, kb2)
        + n3 * NHLOG2PI
        - (s[4] + s[5])
    )
    lp = -4.0 * _log_surface_area(d_w) + n3 * NHLOG2PI - 0.5 * (s[6] + s[7])

    return out, np.asarray(lvp, np.float32), np.asarray(lp, np.float32)
